# revision 5
# baseline (speedup 1.0000x reference)
"""Trainium2 Bass kernel for nn_EquivariantDecoder (GNN message passing).

Sharding: nodes are split into 8 contiguous ranges of 6272 (= 49 tiles of
128); each core owns the edges whose dst lands in its range, so per-node
segment sums are core-local (no collectives). Edges are sorted by dst on
the host and padded so every (core, node-tile) group holds exactly K
tiles of 128 edge slots; the K is baked into the traced program.

Device work per core:
  edge path:  w = silu(m_ij @ W1 + b1) @ W2 + b2   (tile-transposed m_ij)
              scatter-sum of rel*w into the 128-node tile via a one-hot
              matmul (one-hot built on device from dst%128 with iota +
              is_equal; padding slots use col=-1 so they vanish)
  node path:  alpha = silu(h @ vgW1 + vgb1) @ vgW2 + vgb2
              out = sum_k alpha_k * vel_k + scatter_sum * (1/max(cnt,1))
"""

import sys

import numpy as np

try:
    import concourse.bass as bass  # noqa: F401
except Exception:  # pragma: no cover
    sys.path.insert(0, "/opt/trn_rl_repo")

import concourse.bass as bass
import concourse.mybir as mybir
from concourse.bass_utils import run_bass_kernel_spmd
from concourse.tile import TileContext
from concourse.vector_clock import ScopedClock

N_NODES = 50000
N_EDGES = 800000
H = 256
N_CORES = 8
NT = 49                 # node tiles per core
NPC = NT * 128          # 6272 nodes per core
N_PAD = N_CORES * NPC   # 50176
P = 128

# edge-MLP matmul dtype: bfloat16 | float32r | float32
EDGE_DT = mybir.dt.bfloat16
EDGE_NP = mybir.dt.np(EDGE_DT)
F32 = mybir.dt.float32
AF = mybir.ActivationFunctionType
OP = mybir.AluOpType


# ---------------------------------------------------------------------------
# Walrus on this toolchain rejects >2 sync waits on the TileContext tail
# drain ("Too many sync wait commands"); split them across SP NOPs.
def _patched_drain_and_barrier(self, tick_clock, wait_clock):
    drain_inst = self.nc.sync.drain()
    wait_clock.add_sem_waits(
        drain_inst.ins, ScopedClock({None: tick_clock.global_clock})
    )
    si = drain_inst.ins.sync_info
    if si is not None and si.on_wait and len(si.on_wait) > 1:
        extra = list(si.on_wait[1:])
        del si.on_wait[1:]
        for w in extra:
            nop = self.nc.sync.nop(nofuse=True, hint="drain_wait_split")
            nsi = nop.ins.sync_info
            if nsi is None:
                nop.ins.sync_info = mybir.SyncInfo(on_wait=[w], on_update=[])
            else:
                nsi.on_wait.append(w)

    self.nc.all_engine_barrier()
    assert self.sems is not None
    popped = self.nc._tile_sem_poison_stack.pop()
    assert popped is self._sem_poison
    self.nc.clear_and_free_semaphores(list(self.sems.allocated().values()))
    self.nc.all_engine_barrier()


TileContext._drain_and_barrier = _patched_drain_and_barrier


def _split_excess_waits(nc, maxw: int = 1):
    """Walrus rejects >maxw sync waits on one instruction; move the excess
    onto NOPs inserted just before, on the same engine (same-queue program
    order makes this equivalent)."""
    n_split = 0
    for f in nc.m.functions:
        for b in f.blocks:
            out = []
            for inst in b.instructions:
                si = inst.sync_info
                if si is not None and si.on_wait and len(si.on_wait) > maxw:
                    extra = list(si.on_wait[: -maxw])
                    del si.on_wait[: -maxw]
                    for i in range(0, len(extra), maxw):
                        nop = mybir.InstNoOp(
                            name=f"{inst.name}-wsplit{i}",
                            engine=inst.engine,
                            sync_info=mybir.SyncInfo(
                                on_wait=extra[i:i + maxw], on_update=[]),
                            bass_nofuse=True,
                        )
                        out.append(nop)
                    n_split += 1
                out.append(inst)
            b.instructions[:] = out
    return n_split
# ---------------------------------------------------------------------------


def _build_program(K: int, b2: float):
    """Trace the single-core SPMD program for a fixed K (edge tiles per
    node-tile group)."""
    ET = NT * K                      # edge tiles per core
    n_mac = (ET + 3) // 4            # macros of up to 4 edge tiles

    nc = bass.Bass()

    mijT = nc.dram_tensor("mijT", [n_mac, 2, P, 512], EDGE_DT, kind="ExternalInput")
    relw_d = nc.dram_tensor("relw", [P, ET * 4], F32, kind="ExternalInput")
    hT = nc.dram_tensor("hT", [NT, 2, P, P], EDGE_DT, kind="ExternalInput")
    velg_d = nc.dram_tensor("velg", [P, NT * 16], F32, kind="ExternalInput")
    w1b_d = nc.dram_tensor("w1b", [2, 2, P, P], EDGE_DT, kind="ExternalInput")
    w2t_d = nc.dram_tensor("w2t", [2, P, 1], EDGE_DT, kind="ExternalInput")
    b1t_d = nc.dram_tensor("b1t", [2, P, 1], F32, kind="ExternalInput")
    vgw1b_d = nc.dram_tensor("vgw1b", [2, 2, P, P], EDGE_DT, kind="ExternalInput")
    vgw2t_d = nc.dram_tensor("vgw2t", [2, P, 5], EDGE_DT, kind="ExternalInput")
    vgb1t_d = nc.dram_tensor("vgb1t", [2, P, 1], F32, kind="ExternalInput")
    onesb2_d = nc.dram_tensor("onesb2", [1, 133], EDGE_DT, kind="ExternalInput")
    out_d = nc.dram_tensor("out", [NT, P, 3], F32, kind="ExternalOutput")

    with TileContext(nc) as tc:
        with (
            tc.tile_pool(name="const", bufs=1) as cpool,
            tc.tile_pool(name="rhs", bufs=3) as rhs_pool,
            tc.tile_pool(name="s1", bufs=2) as s1_pool,
            tc.tile_pool(name="small", bufs=4) as sm_pool,
            tc.tile_pool(name="oh", bufs=3) as oh_pool,
            tc.tile_pool(name="nodes", bufs=2) as nd_pool,
            tc.tile_pool(name="ps_mm1", bufs=3, space="PSUM") as ps1_pool,
            tc.tile_pool(name="ps_w", bufs=2, space="PSUM") as psw_pool,
            tc.tile_pool(name="ps_sc", bufs=2, space="PSUM") as pssc_pool,
        ):
            # ---- constants ----
            w1 = [[cpool.tile([P, P], EDGE_DT, tag=f"w1_{kk}{hh}", name=f"w1_{kk}{hh}")
                   for hh in range(2)] for kk in range(2)]
            vgw1 = [[cpool.tile([P, P], EDGE_DT, tag=f"vgw1_{kk}{hh}", name=f"vgw1_{kk}{hh}")
                     for hh in range(2)] for kk in range(2)]
            for kk in range(2):
                for hh in range(2):
                    nc.sync.dma_start(w1[kk][hh][:], w1b_d[kk, hh, :, :])
                    nc.sync.dma_start(vgw1[kk][hh][:], vgw1b_d[kk, hh, :, :])
            w2 = [cpool.tile([P, 1], EDGE_DT, tag=f"w2_{hh}", name=f"w2_{hh}") for hh in range(2)]
            b1 = [cpool.tile([P, 1], F32, tag=f"b1_{hh}", name=f"b1_{hh}") for hh in range(2)]
            vgw2 = [cpool.tile([P, 5], EDGE_DT, tag=f"vgw2_{hh}", name=f"vgw2_{hh}") for hh in range(2)]
            vgb1 = [cpool.tile([P, 1], F32, tag=f"vgb1_{hh}", name=f"vgb1_{hh}") for hh in range(2)]
            for hh in range(2):
                nc.sync.dma_start(w2[hh][:], w2t_d[hh, :, :])
                nc.sync.dma_start(b1[hh][:], b1t_d[hh, :, :])
                nc.sync.dma_start(vgw2[hh][:], vgw2t_d[hh, :, :])
                nc.sync.dma_start(vgb1[hh][:], vgb1t_d[hh, :, :])
            onesb2 = cpool.tile([1, 133], EDGE_DT, tag="onesb2")
            nc.sync.dma_start(onesb2[:], onesb2_d[0, :][None, :])

            iota = cpool.tile([P, P], F32, tag="iota")
            nc.gpsimd.iota(iota[:], pattern=[[1, P]], base=0,
                           channel_multiplier=0,
                           allow_small_or_imprecise_dtypes=True)

            relw = cpool.tile([P, ET * 4], F32, tag="relw")
            nc.sync.dma_start(relw[:], relw_d[:, :])
            velg = cpool.tile([P, NT * 16], F32, tag="velg")
            nc.sync.dma_start(velg[:], velg_d[:, :])

            # node-tail emission, called when node tile nt's scatter psum is
            # fully accumulated
            def node_tail(nt: int, ps_sc):
                geom = sm_pool.tile([P, 3], F32, tag="geom")
                nc.vector.tensor_scalar(
                    geom[:], ps_sc[:, 0:3], velg[:, nt * 16 + 15:nt * 16 + 16],
                    None, op0=OP.mult)

                rhn = [nd_pool.tile([P, P], EDGE_DT, tag=f"rhn_{kk}", name=f"rhn_{kk}")
                       for kk in range(2)]
                for kk in range(2):
                    nc.sync.dma_start(rhn[kk][:], hT[nt, kk, :, :])
                psn = [ps1_pool.tile([P, P], F32, tag="ps_mm1", name="psn")
                       for _ in range(2)]
                for hh in range(2):
                    for kk in range(2):
                        nc.tensor.matmul(psn[hh][:], vgw1[kk][hh][:], rhn[kk][:],
                                         start=(kk == 0), stop=(kk == 1))
                s1n = [nd_pool.tile([P, P], EDGE_DT, tag=f"s1n_{hh}", name=f"s1n_{hh}")
                       for hh in range(2)]
                for hh in range(2):
                    nc.scalar.activation(s1n[hh][:], psn[hh][:], AF.Silu,
                                         bias=vgb1[hh][:, 0:1], scale=1.0)
                psa = psw_pool.tile([P, 8], F32, tag="ps_w")
                for hh in range(2):
                    nc.tensor.matmul(psa[:, 0:5], s1n[hh][:], vgw2[hh][:],
                                     start=(hh == 0), stop=False)
                nc.tensor.matmul(psa[:, 0:5], onesb2[:, 0:128],
                                 onesb2[:, 128:133], start=False, stop=True)
                alpha = sm_pool.tile([P, 5], F32, tag="alpha")
                nc.vector.tensor_copy(alpha[:], psa[:, 0:5])

                acc = sm_pool.tile([P, 3], F32, tag="acc")
                tmp = sm_pool.tile([P, 3], F32, tag="tmp")
                vbase = nt * 16
                nc.vector.tensor_scalar(
                    acc[:], velg[:, vbase:vbase + 3], alpha[:, 0:1],
                    None, op0=OP.mult)
                for k5 in range(1, 5):
                    nc.vector.tensor_scalar(
                        tmp[:], velg[:, vbase + 3 * k5:vbase + 3 * k5 + 3],
                        alpha[:, k5:k5 + 1], None, op0=OP.mult)
                    nc.vector.tensor_add(acc[:], acc[:], tmp[:])
                outt = sm_pool.tile([P, 3], F32, tag="outt")
                nc.vector.tensor_add(outt[:], acc[:], geom[:])
                nc.sync.dma_start(out_d[nt, :, :], outt[:])

            # ---- edge-path macro loop ----
            ps_sc = None
            for m in range(n_mac):
                t0 = m * 4
                G = min(4, ET - t0)          # real edge tiles in this macro
                rhs = [rhs_pool.tile([P, 512], EDGE_DT, tag=f"rhs_{kk}", name=f"rhs_{kk}")
                       for kk in range(2)]
                for kk in range(2):
                    nc.sync.dma_start(rhs[kk][:], mijT[m, kk, :, :])
                ps1 = [ps1_pool.tile([P, 512], F32, tag="ps_mm1", name="ps1")
                       for _ in range(2)]
                for hh in range(2):
                    for kk in range(2):
                        nc.tensor.matmul(ps1[hh][:], w1[kk][hh][:], rhs[kk][:],
                                         start=(kk == 0), stop=(kk == 1))
                s1 = [s1_pool.tile([P, 512], EDGE_DT, tag=f"s1_{hh}", name=f"s1_{hh}")
                      for hh in range(2)]
                for hh in range(2):
                    nc.scalar.activation(s1[hh][:], ps1[hh][:], AF.Silu,
                                         bias=b1[hh][:, 0:1], scale=1.0)
                psw = psw_pool.tile([P, 8], F32, tag="ps_w")
                for c in range(G):
                    for hh in range(2):
                        nc.tensor.matmul(psw[:, c:c + 1],
                                         s1[hh][:, c * P:(c + 1) * P],
                                         w2[hh][:],
                                         start=(hh == 0), stop=(hh == 1))
                wpb = sm_pool.tile([P, 4], F32, tag="wpb")
                nc.vector.tensor_scalar(wpb[:, 0:G], psw[:, 0:G], float(b2),
                                        None, op0=OP.add)

                for c in range(G):
                    t = t0 + c
                    nt, j = divmod(t, K)
                    msg = sm_pool.tile([P, 3], F32, tag="msg")
                    nc.vector.tensor_scalar(
                        msg[:], relw[:, 4 * t:4 * t + 3], wpb[:, c:c + 1],
                        None, op0=OP.mult)
                    oh = oh_pool.tile([P, P], F32, tag="oh")
                    nc.vector.tensor_scalar(
                        oh[:], iota[:], relw[:, 4 * t + 3:4 * t + 4],
                        None, op0=OP.is_equal)
                    if j == 0:
                        ps_sc = pssc_pool.tile([P, 3], F32, tag="ps_sc")
                    nc.tensor.matmul(ps_sc[:], oh[:], msg[:],
                                     start=(j == 0), stop=(j == K - 1))
                    if j == K - 1:
                        node_tail(nt, ps_sc)

    _split_excess_waits(nc)
    return nc


def _preprocess(inputs: dict):
    """Shard + lay out all per-core device inputs. Returns (in_maps, K, b2)."""
    h = np.asarray(inputs["h"], np.float32)
    m_ij = np.asarray(inputs["m_ij"], np.float32)
    x = np.asarray(inputs["x"], np.float32)
    vel_all = np.asarray(inputs["vel_all"], np.float32)
    ei = np.asarray(inputs["edge_index"])
    src = ei[0].astype(np.int64)
    dst = ei[1].astype(np.int64)

    counts = np.bincount(dst, minlength=N_NODES).astype(np.float32)
    invc = (1.0 / np.maximum(counts, 1.0)).astype(np.float32)

    order = np.argsort(dst, kind="stable")
    dst_s = dst[order]
    src_s = src[order]
    g = dst_s // P                       # global 128-node group, 0..391
    n_groups = N_PAD // P                # 392
    cg = np.bincount(g, minlength=n_groups)
    K = max(1, int(-(-cg.max() // P)))   # ceil(max group)/128
    ET = NT * K
    n_mac = (ET + 3) // 4
    slots_core = ET * P

    gstart = np.zeros(n_groups, np.int64)
    gstart[1:] = np.cumsum(cg)[:-1]
    within = np.arange(N_EDGES, dtype=np.int64) - gstart[g]
    slot = g * (K * P) + within          # slot in global [392, K*128] layout

    Sg = n_groups * K * P
    colidx = np.full(Sg, -1.0, np.float32)
    colidx[slot] = (dst_s % P).astype(np.float32)
    relp = np.zeros((Sg, 3), np.float32)
    relp[slot] = x[src_s] - x[dst_s]
    mijp = np.zeros((Sg, H), EDGE_NP)
    mijp[slot] = m_ij[order].astype(EDGE_NP)

    # padded node tensors
    hp = np.zeros((N_PAD, H), np.float32)
    hp[:N_NODES] = h
    velp = np.zeros((N_PAD, 5, 3), np.float32)
    velp[:N_NODES] = vel_all
    invp = np.ones(N_PAD, np.float32)
    invp[:N_NODES] = invc

    # weights (shared by all cores)
    w1 = np.asarray(inputs["ew_W1"], np.float32)
    b1 = np.asarray(inputs["ew_b1"], np.float32)
    w2 = np.asarray(inputs["ew_W2"], np.float32)
    b2 = float(np.asarray(inputs["ew_b2"], np.float32)[0])
    vgw1 = np.asarray(inputs["vg_W1"], np.float32)
    vgb1 = np.asarray(inputs["vg_b1"], np.float32)
    vgw2 = np.asarray(inputs["vg_W2"], np.float32)
    vgb2 = np.asarray(inputs["vg_b2"], np.float32)

    w1b = w1.reshape(2, P, 2, P).transpose(0, 2, 1, 3).astype(EDGE_NP).copy()
    w2t = w2.reshape(2, P, 1).astype(EDGE_NP).copy()
    b1t = b1.reshape(2, P, 1).copy()
    vgw1b = vgw1.reshape(2, P, 2, P).transpose(0, 2, 1, 3).astype(EDGE_NP).copy()
    vgw2t = vgw2.reshape(2, P, 5).astype(EDGE_NP).copy()
    vgb1t = vgb1.reshape(2, P, 1).copy()
    onesb2 = np.zeros((1, 133), EDGE_NP)
    onesb2[0, :P] = 1.0
    onesb2[0, P:P + 5] = vgb2.astype(EDGE_NP)

    mijp = mijp.reshape(N_CORES, ET, P, H)
    relp = relp.reshape(N_CORES, ET, P, 3)
    colidx = colidx.reshape(N_CORES, ET, P)

    in_maps = []
    for k in range(N_CORES):
        # mijT: [n_mac, 2, 128, 512]; tile block = m_ij tile transposed
        b = mijp[k].transpose(0, 2, 1).reshape(ET, 2, P, P)
        full = np.zeros((n_mac * 4, 2, P, P), EDGE_NP)
        full[:ET] = b
        mijT = np.ascontiguousarray(
            full.reshape(n_mac, 4, 2, P, P).transpose(0, 2, 3, 1, 4)
        ).reshape(n_mac, 2, P, 512)

        rw = np.empty((P, ET, 4), np.float32)
        rw[:, :, 0:3] = relp[k].transpose(1, 0, 2)
        rw[:, :, 3] = colidx[k].T
        relw = np.ascontiguousarray(rw).reshape(P, ET * 4)

        hk = hp[k * NPC:(k + 1) * NPC].reshape(NT, P, H)
        hTk = np.ascontiguousarray(
            hk.transpose(0, 2, 1).reshape(NT, 2, P, P).astype(EDGE_NP))

        vg = np.empty((P, NT, 16), np.float32)
        vg[:, :, 0:15] = (velp[k * NPC:(k + 1) * NPC]
                          .reshape(NT, P, 15).transpose(1, 0, 2))
        vg[:, :, 15] = invp[k * NPC:(k + 1) * NPC].reshape(NT, P).T
        velg = np.ascontiguousarray(vg).reshape(P, NT * 16)

        in_maps.append({
            "mijT": mijT,
            "relw": relw,
            "hT": hTk,
            "velg": velg,
            "w1b": w1b,
            "w2t": w2t,
            "b1t": b1t,
            "vgw1b": vgw1b,
            "vgw2t": vgw2t,
            "vgb1t": vgb1t,
            "onesb2": onesb2,
        })
    return in_maps, K, b2


def kernel(**inputs) -> np.ndarray:
    in_maps, K, b2 = _preprocess(inputs)
    nc = _build_program(K, b2)
    res = run_bass_kernel_spmd(nc, in_maps, list(range(N_CORES)))
    parts = [res.results[k]["out"].reshape(NPC, 3) for k in range(N_CORES)]
    return np.concatenate(parts, axis=0)[:N_NODES].astype(np.float32)


# revision 8
# speedup vs baseline: 1.4468x; 1.4468x over previous
"""Trainium2 Bass kernel for nn_EquivariantDecoder (GNN message passing).

Sharding: nodes are split into 8 contiguous ranges of 6272 (= 49 tiles of
128); each core owns the edges whose dst lands in its range, so per-node
segment sums are core-local (no collectives). Edges are sorted by dst on
the host and padded so every (core, node-tile) group holds exactly K
tiles of 128 edge slots; the K is baked into the traced program.

Device work per core:
  edge path:  w = silu(m_ij @ W1 + b1) @ W2 + b2   (tile-transposed m_ij)
              scatter-sum of rel*w into the 128-node tile via a one-hot
              matmul (one-hot built on device from dst%128 with iota +
              is_equal; padding slots use col=-1 so they vanish)
  node path:  alpha = silu(h @ vgW1 + vgb1) @ vgW2 + vgb2
              out = sum_k alpha_k * vel_k + scatter_sum * (1/max(cnt,1))
"""

import sys

import numpy as np

try:
    import concourse.bass as bass  # noqa: F401
except Exception:  # pragma: no cover
    sys.path.insert(0, "/opt/trn_rl_repo")

import concourse.bass as bass
import concourse.mybir as mybir
from concourse.bass_utils import run_bass_kernel_spmd
from concourse.tile import TileContext
from concourse.vector_clock import ScopedClock

N_NODES = 50000
N_EDGES = 800000
H = 256
N_CORES = 8
NT = 49                 # node tiles per core
NPC = NT * 128          # 6272 nodes per core
N_PAD = N_CORES * NPC   # 50176
P = 128

# edge-MLP matmul dtype: bfloat16 | float32r | float32
EDGE_DT = mybir.dt.bfloat16
EDGE_NP = mybir.dt.np(EDGE_DT)
F32 = mybir.dt.float32
AF = mybir.ActivationFunctionType
OP = mybir.AluOpType


# ---------------------------------------------------------------------------
# Walrus on this toolchain rejects >2 sync waits on the TileContext tail
# drain ("Too many sync wait commands"); split them across SP NOPs.
def _patched_drain_and_barrier(self, tick_clock, wait_clock):
    drain_inst = self.nc.sync.drain()
    wait_clock.add_sem_waits(
        drain_inst.ins, ScopedClock({None: tick_clock.global_clock})
    )
    si = drain_inst.ins.sync_info
    if si is not None and si.on_wait and len(si.on_wait) > 1:
        extra = list(si.on_wait[1:])
        del si.on_wait[1:]
        for w in extra:
            nop = self.nc.sync.nop(nofuse=True, hint="drain_wait_split")
            nsi = nop.ins.sync_info
            if nsi is None:
                nop.ins.sync_info = mybir.SyncInfo(on_wait=[w], on_update=[])
            else:
                nsi.on_wait.append(w)

    self.nc.all_engine_barrier()
    assert self.sems is not None
    popped = self.nc._tile_sem_poison_stack.pop()
    assert popped is self._sem_poison
    self.nc.clear_and_free_semaphores(list(self.sems.allocated().values()))
    self.nc.all_engine_barrier()


TileContext._drain_and_barrier = _patched_drain_and_barrier


def _split_excess_waits(nc, maxw: int = 1):
    """Walrus rejects >maxw sync waits on one instruction; move the excess
    onto NOPs inserted just before, on the same engine (same-queue program
    order makes this equivalent)."""
    n_split = 0
    for f in nc.m.functions:
        for b in f.blocks:
            out = []
            for inst in b.instructions:
                si = inst.sync_info
                if si is not None and si.on_wait and len(si.on_wait) > maxw:
                    extra = list(si.on_wait[: -maxw])
                    del si.on_wait[: -maxw]
                    for i in range(0, len(extra), maxw):
                        nop = mybir.InstNoOp(
                            name=f"{inst.name}-wsplit{i}",
                            engine=inst.engine,
                            sync_info=mybir.SyncInfo(
                                on_wait=extra[i:i + maxw], on_update=[]),
                            bass_nofuse=True,
                        )
                        out.append(nop)
                    n_split += 1
                out.append(inst)
            b.instructions[:] = out
    return n_split
# ---------------------------------------------------------------------------


def _build_program(K: int, b2: float):
    """Trace the single-core SPMD program for a fixed K (edge tiles per
    node-tile group)."""
    ET = NT * K                      # edge tiles per core
    n_mac = (ET + 3) // 4            # macros of up to 4 edge tiles

    nc = bass.Bass()

    mijT = nc.dram_tensor("mijT", [n_mac, 2, P, 512], EDGE_DT, kind="ExternalInput")
    relw_d = nc.dram_tensor("relw", [P, ET * 4], F32, kind="ExternalInput")
    hT = nc.dram_tensor("hT", [NT, 2, P, P], EDGE_DT, kind="ExternalInput")
    velg_d = nc.dram_tensor("velg", [P, NT * 16], F32, kind="ExternalInput")
    w1b_d = nc.dram_tensor("w1b", [2, 2, P, P], EDGE_DT, kind="ExternalInput")
    w2t_d = nc.dram_tensor("w2t", [2, P, 1], EDGE_DT, kind="ExternalInput")
    b1t_d = nc.dram_tensor("b1t", [2, P, 1], F32, kind="ExternalInput")
    vgw1b_d = nc.dram_tensor("vgw1b", [2, 2, P, P], EDGE_DT, kind="ExternalInput")
    vgw2t_d = nc.dram_tensor("vgw2t", [2, P, 5], EDGE_DT, kind="ExternalInput")
    vgb1t_d = nc.dram_tensor("vgb1t", [2, P, 1], F32, kind="ExternalInput")
    onesb2_d = nc.dram_tensor("onesb2", [1, 133], EDGE_DT, kind="ExternalInput")
    out_d = nc.dram_tensor("out", [NT, P, 3], F32, kind="ExternalOutput")

    with TileContext(nc) as tc:
        with (
            tc.tile_pool(name="const", bufs=1) as cpool,
            tc.tile_pool(name="rhs", bufs=3) as rhs_pool,
            tc.tile_pool(name="s1", bufs=2) as s1_pool,
            tc.tile_pool(name="small", bufs=4) as sm_pool,
            tc.tile_pool(name="oh", bufs=3) as oh_pool,
            tc.tile_pool(name="nodes", bufs=2) as nd_pool,
            tc.tile_pool(name="ps_mm1", bufs=3, space="PSUM") as ps1_pool,
            tc.tile_pool(name="ps_w", bufs=2, space="PSUM") as psw_pool,
            tc.tile_pool(name="ps_sc", bufs=2, space="PSUM") as pssc_pool,
        ):
            # ---- constants ----
            w1 = [[cpool.tile([P, P], EDGE_DT, tag=f"w1_{kk}{hh}", name=f"w1_{kk}{hh}")
                   for hh in range(2)] for kk in range(2)]
            vgw1 = [[cpool.tile([P, P], EDGE_DT, tag=f"vgw1_{kk}{hh}", name=f"vgw1_{kk}{hh}")
                     for hh in range(2)] for kk in range(2)]
            for kk in range(2):
                for hh in range(2):
                    nc.sync.dma_start(w1[kk][hh][:], w1b_d[kk, hh, :, :])
                    nc.sync.dma_start(vgw1[kk][hh][:], vgw1b_d[kk, hh, :, :])
            w2 = [cpool.tile([P, 1], EDGE_DT, tag=f"w2_{hh}", name=f"w2_{hh}") for hh in range(2)]
            b1 = [cpool.tile([P, 1], F32, tag=f"b1_{hh}", name=f"b1_{hh}") for hh in range(2)]
            vgw2 = [cpool.tile([P, 5], EDGE_DT, tag=f"vgw2_{hh}", name=f"vgw2_{hh}") for hh in range(2)]
            vgb1 = [cpool.tile([P, 1], F32, tag=f"vgb1_{hh}", name=f"vgb1_{hh}") for hh in range(2)]
            for hh in range(2):
                nc.sync.dma_start(w2[hh][:], w2t_d[hh, :, :])
                nc.sync.dma_start(b1[hh][:], b1t_d[hh, :, :])
                nc.sync.dma_start(vgw2[hh][:], vgw2t_d[hh, :, :])
                nc.sync.dma_start(vgb1[hh][:], vgb1t_d[hh, :, :])
            onesb2 = cpool.tile([1, 133], EDGE_DT, tag="onesb2")
            nc.sync.dma_start(onesb2[:], onesb2_d[0, :][None, :])

            iota = cpool.tile([P, P], EDGE_DT, tag="iota")
            nc.gpsimd.iota(iota[:], pattern=[[1, P]], base=0,
                           channel_multiplier=0,
                           allow_small_or_imprecise_dtypes=True)

            relw = cpool.tile([P, ET * 4], F32, tag="relw")
            nc.sync.dma_start(relw[:], relw_d[:, :])
            velg = cpool.tile([P, NT * 16], F32, tag="velg")
            nc.sync.dma_start(velg[:], velg_d[:, :])

            # node-tail emission, called when node tile nt's scatter psum is
            # fully accumulated
            def node_tail(nt: int, ps_sc):
                geom = sm_pool.tile([P, 3], F32, tag="geom")
                nc.vector.tensor_scalar(
                    geom[:], ps_sc[:, 0:3], velg[:, nt * 16 + 15:nt * 16 + 16],
                    None, op0=OP.mult)

                rhn = [nd_pool.tile([P, P], EDGE_DT, tag=f"rhn_{kk}", name=f"rhn_{kk}")
                       for kk in range(2)]
                for kk in range(2):
                    nc.sync.dma_start(rhn[kk][:], hT[nt, kk, :, :])
                psn = [ps1_pool.tile([P, P], F32, tag="ps_mm1", name="psn")
                       for _ in range(2)]
                for hh in range(2):
                    for kk in range(2):
                        nc.tensor.matmul(psn[hh][:], vgw1[kk][hh][:], rhn[kk][:],
                                         start=(kk == 0), stop=(kk == 1))
                s1n = [nd_pool.tile([P, P], EDGE_DT, tag=f"s1n_{hh}", name=f"s1n_{hh}")
                       for hh in range(2)]
                for hh in range(2):
                    nc.scalar.activation(s1n[hh][:], psn[hh][:], AF.Silu,
                                         bias=vgb1[hh][:, 0:1], scale=1.0)
                psa = psw_pool.tile([P, 8], F32, tag="ps_w")
                for hh in range(2):
                    nc.tensor.matmul(psa[:, 0:5], s1n[hh][:], vgw2[hh][:],
                                     start=(hh == 0), stop=False)
                nc.tensor.matmul(psa[:, 0:5], onesb2[:, 0:128],
                                 onesb2[:, 128:133], start=False, stop=True)
                alpha = sm_pool.tile([P, 5], F32, tag="alpha")
                nc.vector.tensor_copy(alpha[:], psa[:, 0:5])

                acc = sm_pool.tile([P, 3], F32, tag="acc")
                tmp = sm_pool.tile([P, 3], F32, tag="tmp")
                vbase = nt * 16
                nc.vector.tensor_scalar(
                    acc[:], velg[:, vbase:vbase + 3], alpha[:, 0:1],
                    None, op0=OP.mult)
                for k5 in range(1, 5):
                    nc.vector.tensor_scalar(
                        tmp[:], velg[:, vbase + 3 * k5:vbase + 3 * k5 + 3],
                        alpha[:, k5:k5 + 1], None, op0=OP.mult)
                    nc.vector.tensor_add(acc[:], acc[:], tmp[:])
                outt = sm_pool.tile([P, 3], F32, tag="outt")
                nc.vector.tensor_add(outt[:], acc[:], geom[:])
                nc.sync.dma_start(out_d[nt, :, :], outt[:])

            # ---- edge-path macro loop ----
            ps_sc = None
            for m in range(n_mac):
                t0 = m * 4
                G = min(4, ET - t0)          # real edge tiles in this macro
                rhs = [rhs_pool.tile([P, 512], EDGE_DT, tag=f"rhs_{kk}", name=f"rhs_{kk}")
                       for kk in range(2)]
                for kk in range(2):
                    nc.sync.dma_start(rhs[kk][:], mijT[m, kk, :, :])
                ps1 = [ps1_pool.tile([P, 512], F32, tag="ps_mm1", name="ps1")
                       for _ in range(2)]
                for hh in range(2):
                    for kk in range(2):
                        nc.tensor.matmul(ps1[hh][:], w1[kk][hh][:], rhs[kk][:],
                                         start=(kk == 0), stop=(kk == 1))
                s1 = [s1_pool.tile([P, 512], EDGE_DT, tag=f"s1_{hh}", name=f"s1_{hh}")
                      for hh in range(2)]
                for hh in range(2):
                    nc.scalar.activation(s1[hh][:], ps1[hh][:], AF.Silu,
                                         bias=b1[hh][:, 0:1], scale=1.0)
                psw = psw_pool.tile([P, 8], F32, tag="ps_w")
                for c in range(G):
                    for hh in range(2):
                        nc.tensor.matmul(psw[:, c:c + 1],
                                         s1[hh][:, c * P:(c + 1) * P],
                                         w2[hh][:],
                                         start=(hh == 0), stop=(hh == 1))
                wpb = sm_pool.tile([P, 4], F32, tag="wpb")
                nc.vector.tensor_scalar(wpb[:, 0:G], psw[:, 0:G], float(b2),
                                        None, op0=OP.add)

                for c in range(G):
                    t = t0 + c
                    nt, j = divmod(t, K)
                    msg = sm_pool.tile([P, 3], EDGE_DT, tag="msg")
                    nc.vector.tensor_scalar(
                        msg[:], relw[:, 4 * t:4 * t + 3], wpb[:, c:c + 1],
                        None, op0=OP.mult)
                    oh = oh_pool.tile([P, P], EDGE_DT, tag="oh")
                    nc.vector.tensor_scalar(
                        oh[:], iota[:], relw[:, 4 * t + 3:4 * t + 4],
                        None, op0=OP.is_equal)
                    if j == 0:
                        ps_sc = pssc_pool.tile([P, 3], F32, tag="ps_sc")
                    nc.tensor.matmul(ps_sc[:], oh[:], msg[:],
                                     start=(j == 0), stop=(j == K - 1))
                    if j == K - 1:
                        node_tail(nt, ps_sc)

    _split_excess_waits(nc)
    return nc


def _preprocess(inputs: dict):
    """Shard + lay out all per-core device inputs. Returns (in_maps, K, b2)."""
    h = np.asarray(inputs["h"], np.float32)
    m_ij = np.asarray(inputs["m_ij"], np.float32)
    x = np.asarray(inputs["x"], np.float32)
    vel_all = np.asarray(inputs["vel_all"], np.float32)
    ei = np.asarray(inputs["edge_index"])
    src = ei[0].astype(np.int64)
    dst = ei[1].astype(np.int64)

    counts = np.bincount(dst, minlength=N_NODES).astype(np.float32)
    invc = (1.0 / np.maximum(counts, 1.0)).astype(np.float32)

    order = np.argsort(dst, kind="stable")
    dst_s = dst[order]
    src_s = src[order]
    g = dst_s // P                       # global 128-node group, 0..391
    n_groups = N_PAD // P                # 392
    cg = np.bincount(g, minlength=n_groups)
    K = max(1, int(-(-cg.max() // P)))   # ceil(max group)/128
    ET = NT * K
    n_mac = (ET + 3) // 4
    slots_core = ET * P

    gstart = np.zeros(n_groups, np.int64)
    gstart[1:] = np.cumsum(cg)[:-1]
    within = np.arange(N_EDGES, dtype=np.int64) - gstart[g]
    slot = g * (K * P) + within          # slot in global [392, K*128] layout

    Sg = n_groups * K * P
    colidx = np.full(Sg, -1.0, np.float32)
    colidx[slot] = (dst_s % P).astype(np.float32)
    relp = np.zeros((Sg, 3), np.float32)
    relp[slot] = x[src_s] - x[dst_s]
    mijp = np.zeros((Sg, H), EDGE_NP)
    mijp[slot] = m_ij[order].astype(EDGE_NP)

    # padded node tensors
    hp = np.zeros((N_PAD, H), np.float32)
    hp[:N_NODES] = h
    velp = np.zeros((N_PAD, 5, 3), np.float32)
    velp[:N_NODES] = vel_all
    invp = np.ones(N_PAD, np.float32)
    invp[:N_NODES] = invc

    # weights (shared by all cores)
    w1 = np.asarray(inputs["ew_W1"], np.float32)
    b1 = np.asarray(inputs["ew_b1"], np.float32)
    w2 = np.asarray(inputs["ew_W2"], np.float32)
    b2 = float(np.asarray(inputs["ew_b2"], np.float32)[0])
    vgw1 = np.asarray(inputs["vg_W1"], np.float32)
    vgb1 = np.asarray(inputs["vg_b1"], np.float32)
    vgw2 = np.asarray(inputs["vg_W2"], np.float32)
    vgb2 = np.asarray(inputs["vg_b2"], np.float32)

    w1b = w1.reshape(2, P, 2, P).transpose(0, 2, 1, 3).astype(EDGE_NP).copy()
    w2t = w2.reshape(2, P, 1).astype(EDGE_NP).copy()
    b1t = b1.reshape(2, P, 1).copy()
    vgw1b = vgw1.reshape(2, P, 2, P).transpose(0, 2, 1, 3).astype(EDGE_NP).copy()
    vgw2t = vgw2.reshape(2, P, 5).astype(EDGE_NP).copy()
    vgb1t = vgb1.reshape(2, P, 1).copy()
    onesb2 = np.zeros((1, 133), EDGE_NP)
    onesb2[0, :P] = 1.0
    onesb2[0, P:P + 5] = vgb2.astype(EDGE_NP)

    mijp = mijp.reshape(N_CORES, ET, P, H)
    relp = relp.reshape(N_CORES, ET, P, 3)
    colidx = colidx.reshape(N_CORES, ET, P)

    in_maps = []
    for k in range(N_CORES):
        # mijT: [n_mac, 2, 128, 512]; tile block = m_ij tile transposed
        b = mijp[k].transpose(0, 2, 1).reshape(ET, 2, P, P)
        full = np.zeros((n_mac * 4, 2, P, P), EDGE_NP)
        full[:ET] = b
        mijT = np.ascontiguousarray(
            full.reshape(n_mac, 4, 2, P, P).transpose(0, 2, 3, 1, 4)
        ).reshape(n_mac, 2, P, 512)

        rw = np.empty((P, ET, 4), np.float32)
        rw[:, :, 0:3] = relp[k].transpose(1, 0, 2)
        rw[:, :, 3] = colidx[k].T
        relw = np.ascontiguousarray(rw).reshape(P, ET * 4)

        hk = hp[k * NPC:(k + 1) * NPC].reshape(NT, P, H)
        hTk = np.ascontiguousarray(
            hk.transpose(0, 2, 1).reshape(NT, 2, P, P).astype(EDGE_NP))

        vg = np.empty((P, NT, 16), np.float32)
        vg[:, :, 0:15] = (velp[k * NPC:(k + 1) * NPC]
                          .reshape(NT, P, 15).transpose(1, 0, 2))
        vg[:, :, 15] = invp[k * NPC:(k + 1) * NPC].reshape(NT, P).T
        velg = np.ascontiguousarray(vg).reshape(P, NT * 16)

        in_maps.append({
            "mijT": mijT,
            "relw": relw,
            "hT": hTk,
            "velg": velg,
            "w1b": w1b,
            "w2t": w2t,
            "b1t": b1t,
            "vgw1b": vgw1b,
            "vgw2t": vgw2t,
            "vgb1t": vgb1t,
            "onesb2": onesb2,
        })
    return in_maps, K, b2


def kernel(**inputs) -> np.ndarray:
    in_maps, K, b2 = _preprocess(inputs)
    nc = _build_program(K, b2)
    res = run_bass_kernel_spmd(nc, in_maps, list(range(N_CORES)))
    parts = [res.results[k]["out"].reshape(NPC, 3) for k in range(N_CORES)]
    return np.concatenate(parts, axis=0)[:N_NODES].astype(np.float32)


# revision 17
# speedup vs baseline: 1.7053x; 1.1787x over previous
"""Trainium2 Bass kernel for nn_EquivariantDecoder (GNN message passing).

Sharding: nodes are split into 8 contiguous ranges of 6272 (= 49 tiles of
128); each core owns the edges whose dst lands in its range, so per-node
segment sums are core-local (no collectives). Edges are sorted by dst on
the host and padded so every (core, node-tile) group holds exactly K
tiles of 128 edge slots; the K is baked into the traced program.

Device work per core:
  edge path:  w = silu(m_ij @ W1 + b1) @ W2 + b2   (tile-transposed m_ij)
              scatter-sum of rel*w into the 128-node tile via a one-hot
              matmul (one-hot built on device from dst%128 with iota +
              is_equal; padding slots use col=-1 so they vanish)
  node path:  alpha = silu(h @ vgW1 + vgb1) @ vgW2 + vgb2
              out = sum_k alpha_k * vel_k + scatter_sum * (1/max(cnt,1))
"""

import sys

import numpy as np

try:
    import concourse.bass as bass  # noqa: F401
except Exception:  # pragma: no cover
    sys.path.insert(0, "/opt/trn_rl_repo")

import concourse.bass as bass
import concourse.mybir as mybir
from concourse.bass_utils import run_bass_kernel_spmd
from concourse.tile import TileContext
from concourse.vector_clock import ScopedClock

N_NODES = 50000
N_EDGES = 800000
H = 256
N_CORES = 8
NT = 49                 # node tiles per core
NPC = NT * 128          # 6272 nodes per core
N_PAD = N_CORES * NPC   # 50176
P = 128

# edge-MLP matmul dtype: bfloat16 | float32r | float32
EDGE_DT = mybir.dt.bfloat16
EDGE_NP = mybir.dt.np(EDGE_DT)
F32 = mybir.dt.float32
AF = mybir.ActivationFunctionType
OP = mybir.AluOpType


# ---------------------------------------------------------------------------
# Walrus on this toolchain rejects >2 sync waits on the TileContext tail
# drain ("Too many sync wait commands"); split them across SP NOPs.
def _patched_drain_and_barrier(self, tick_clock, wait_clock):
    drain_inst = self.nc.sync.drain()
    wait_clock.add_sem_waits(
        drain_inst.ins, ScopedClock({None: tick_clock.global_clock})
    )
    si = drain_inst.ins.sync_info
    if si is not None and si.on_wait and len(si.on_wait) > 1:
        extra = list(si.on_wait[1:])
        del si.on_wait[1:]
        for w in extra:
            nop = self.nc.sync.nop(nofuse=True, hint="drain_wait_split")
            nsi = nop.ins.sync_info
            if nsi is None:
                nop.ins.sync_info = mybir.SyncInfo(on_wait=[w], on_update=[])
            else:
                nsi.on_wait.append(w)

    self.nc.all_engine_barrier()
    assert self.sems is not None
    popped = self.nc._tile_sem_poison_stack.pop()
    assert popped is self._sem_poison
    self.nc.clear_and_free_semaphores(list(self.sems.allocated().values()))
    self.nc.all_engine_barrier()


TileContext._drain_and_barrier = _patched_drain_and_barrier


def _split_excess_waits(nc, maxw: int = 1):
    """Walrus rejects >maxw sync waits on one instruction; move the excess
    onto NOPs inserted just before, on the same engine (same-queue program
    order makes this equivalent)."""
    n_split = 0
    for f in nc.m.functions:
        for b in f.blocks:
            out = []
            for inst in b.instructions:
                si = inst.sync_info
                if si is not None and si.on_wait and len(si.on_wait) > maxw:
                    extra = list(si.on_wait[: -maxw])
                    del si.on_wait[: -maxw]
                    for i in range(0, len(extra), maxw):
                        nop = mybir.InstNoOp(
                            name=f"{inst.name}-wsplit{i}",
                            engine=inst.engine,
                            sync_info=mybir.SyncInfo(
                                on_wait=extra[i:i + maxw], on_update=[]),
                            bass_nofuse=True,
                        )
                        out.append(nop)
                    n_split += 1
                out.append(inst)
            b.instructions[:] = out
    return n_split
# ---------------------------------------------------------------------------


def _build_program(K: int, b2: float):
    """Trace the single-core SPMD program for a fixed K (edge tiles per
    node-tile group)."""
    ET = NT * K                      # edge tiles per core
    n_mac = (ET + 3) // 4            # macros of up to 4 edge tiles
    n_sup = (n_mac + 3) // 4         # supertiles of 4 macros (1 DMA each)

    nc = bass.Bass()

    mijT = nc.dram_tensor("mijT", [n_sup, P, 4096], EDGE_DT, kind="ExternalInput")
    relw_d = nc.dram_tensor("relw", [P, ET * 4], F32, kind="ExternalInput")
    hT = nc.dram_tensor("hT", [NT, P, 2 * P], EDGE_DT, kind="ExternalInput")
    velg_d = nc.dram_tensor("velg", [P, NT * 16], F32, kind="ExternalInput")
    w1b_d = nc.dram_tensor("w1b", [2, 2, P, P], EDGE_DT, kind="ExternalInput")
    w2t_d = nc.dram_tensor("w2t", [2, P, 1], EDGE_DT, kind="ExternalInput")
    b1t_d = nc.dram_tensor("b1t", [2, P, 1], F32, kind="ExternalInput")
    vgw1b_d = nc.dram_tensor("vgw1b", [2, 2, P, P], EDGE_DT, kind="ExternalInput")
    vgw2t_d = nc.dram_tensor("vgw2t", [2, P, 5], EDGE_DT, kind="ExternalInput")
    vgb1t_d = nc.dram_tensor("vgb1t", [2, P, 1], F32, kind="ExternalInput")
    onesb2_d = nc.dram_tensor("onesb2", [1, 133], EDGE_DT, kind="ExternalInput")
    out_d = nc.dram_tensor("out", [P, NT * 3], F32, kind="ExternalOutput")

    with TileContext(nc) as tc:
        with (
            tc.tile_pool(name="const", bufs=1) as cpool,
            tc.tile_pool(name="rhs", bufs=3) as rhs_pool,
            tc.tile_pool(name="s1", bufs=2) as s1_pool,
            tc.tile_pool(name="small", bufs=4) as sm_pool,
            tc.tile_pool(name="oh", bufs=3) as oh_pool,
            tc.tile_pool(name="nodes", bufs=3) as nd_pool,
            tc.tile_pool(name="ps_mm1", bufs=3, space="PSUM") as ps1_pool,
            tc.tile_pool(name="ps_w", bufs=2, space="PSUM") as psw_pool,
            tc.tile_pool(name="ps_sc", bufs=2, space="PSUM") as pssc_pool,
        ):
            # ---- constants ----
            w1 = [[cpool.tile([P, P], EDGE_DT, tag=f"w1_{kk}{hh}", name=f"w1_{kk}{hh}")
                   for hh in range(2)] for kk in range(2)]
            vgw1 = [[cpool.tile([P, P], EDGE_DT, tag=f"vgw1_{kk}{hh}", name=f"vgw1_{kk}{hh}")
                     for hh in range(2)] for kk in range(2)]
            for kk in range(2):
                for hh in range(2):
                    nc.sync.dma_start(w1[kk][hh][:], w1b_d[kk, hh, :, :])
                    nc.sync.dma_start(vgw1[kk][hh][:], vgw1b_d[kk, hh, :, :])
            w2 = [cpool.tile([P, 1], EDGE_DT, tag=f"w2_{hh}", name=f"w2_{hh}") for hh in range(2)]
            b1 = [cpool.tile([P, 1], F32, tag=f"b1_{hh}", name=f"b1_{hh}") for hh in range(2)]
            vgw2 = [cpool.tile([P, 5], EDGE_DT, tag=f"vgw2_{hh}", name=f"vgw2_{hh}") for hh in range(2)]
            vgb1 = [cpool.tile([P, 1], F32, tag=f"vgb1_{hh}", name=f"vgb1_{hh}") for hh in range(2)]
            for hh in range(2):
                nc.sync.dma_start(w2[hh][:], w2t_d[hh, :, :])
                nc.sync.dma_start(b1[hh][:], b1t_d[hh, :, :])
                nc.sync.dma_start(vgw2[hh][:], vgw2t_d[hh, :, :])
                nc.sync.dma_start(vgb1[hh][:], vgb1t_d[hh, :, :])
            onesb2 = cpool.tile([1, 133], EDGE_DT, tag="onesb2")
            nc.sync.dma_start(onesb2[:], onesb2_d[0, :][None, :])

            # iota values 0..127 repeated 4x: one-hot build for a whole macro
            iota = cpool.tile([P, 4 * P], EDGE_DT, tag="iota")
            nc.gpsimd.iota(iota[:], pattern=[[0, 4], [1, P]], base=0,
                           channel_multiplier=0,
                           allow_small_or_imprecise_dtypes=True)

            relw = cpool.tile([P, ET * 4], F32, tag="relw")
            nc.scalar.dma_start(relw[:], relw_d[:, :])
            velg = cpool.tile([P, NT * 16], F32, tag="velg")
            nc.scalar.dma_start(velg[:], velg_d[:, :])

            # packed output, one column triple per node tile; single DMA at end
            outbuf = cpool.tile([P, NT * 3], F32, tag="outbuf")

            # node-tail emission, called when node tile nt's scatter psum is
            # fully accumulated
            def node_tail(nt: int, ps_sc):
                geom = sm_pool.tile([P, 3], F32, tag="geom")
                nc.vector.tensor_scalar(
                    geom[:], ps_sc[:, 0:3], velg[:, nt * 16 + 15:nt * 16 + 16],
                    None, op0=OP.mult)

                rhn = nd_pool.tile([P, 2 * P], EDGE_DT, tag="rhn")
                eng = nc.sync if nt % 2 == 0 else nc.scalar
                eng.dma_start(rhn[:], hT[nt, :, :])
                psn = [ps1_pool.tile([P, P], F32, tag="ps_mm1", name="psn")
                       for _ in range(2)]
                for hh in range(2):
                    for kk in range(2):
                        nc.tensor.matmul(psn[hh][:], vgw1[kk][hh][:],
                                         rhn[:, kk * P:(kk + 1) * P],
                                         start=(kk == 0), stop=(kk == 1))
                s1n = [nd_pool.tile([P, P], EDGE_DT, tag=f"s1n_{hh}", name=f"s1n_{hh}")
                       for hh in range(2)]
                for hh in range(2):
                    nc.scalar.activation(s1n[hh][:], psn[hh][:], AF.Silu,
                                         bias=vgb1[hh][:, 0:1], scale=1.0)
                psa = psw_pool.tile([P, 8], F32, tag="ps_w")
                for hh in range(2):
                    nc.tensor.matmul(psa[:, 0:5], s1n[hh][:], vgw2[hh][:],
                                     start=(hh == 0), stop=False)
                nc.tensor.matmul(psa[:, 0:5], onesb2[:, 0:128],
                                 onesb2[:, 128:133], start=False, stop=True)
                alpha = sm_pool.tile([P, 5], F32, tag="alpha")
                nc.vector.tensor_copy(alpha[:], psa[:, 0:5])

                acc = sm_pool.tile([P, 3], F32, tag="acc")
                tmp = sm_pool.tile([P, 3], F32, tag="tmp")
                vbase = nt * 16
                nc.vector.tensor_scalar(
                    acc[:], velg[:, vbase:vbase + 3], alpha[:, 0:1],
                    None, op0=OP.mult)
                for k5 in range(1, 5):
                    nc.vector.tensor_scalar(
                        tmp[:], velg[:, vbase + 3 * k5:vbase + 3 * k5 + 3],
                        alpha[:, k5:k5 + 1], None, op0=OP.mult)
                    nc.vector.tensor_add(acc[:], acc[:], tmp[:])
                nc.vector.tensor_add(outbuf[:, nt * 3:nt * 3 + 3],
                                     acc[:], geom[:])

            # ---- edge-path macro loop ----
            ps_sc = None
            sup = None
            for m in range(n_mac):
                t0 = m * 4
                G = min(4, ET - t0)          # real edge tiles in this macro
                si, sm = divmod(m, 4)
                if sm == 0:
                    sup = rhs_pool.tile([P, 4096], EDGE_DT, tag="sup",
                                        name="sup")
                    eng = nc.sync if si % 2 == 0 else nc.scalar
                    eng.dma_start(sup[:], mijT[si, :, :])
                rhs = sup[:, sm * 1024:(sm + 1) * 1024]
                ps1 = [ps1_pool.tile([P, 512], F32, tag="ps_mm1", name="ps1")
                       for _ in range(2)]
                for hh in range(2):
                    for kk in range(2):
                        nc.tensor.matmul(ps1[hh][:], w1[kk][hh][:],
                                         rhs[:, kk * 512:(kk + 1) * 512],
                                         start=(kk == 0), stop=(kk == 1))
                s1 = [s1_pool.tile([P, 512], EDGE_DT, tag=f"s1_{hh}", name=f"s1_{hh}")
                      for hh in range(2)]
                for hh in range(2):
                    nc.scalar.activation(s1[hh][:], ps1[hh][:], AF.Silu,
                                         bias=b1[hh][:, 0:1], scale=1.0)
                psw = psw_pool.tile([P, 8], F32, tag="ps_w")
                for c in range(G):
                    for hh in range(2):
                        nc.tensor.matmul(psw[:, c:c + 1],
                                         s1[hh][:, c * P:(c + 1) * P],
                                         w2[hh][:],
                                         start=(hh == 0), stop=(hh == 1))
                wpb = sm_pool.tile([P, 4], F32, tag="wpb")
                nc.vector.tensor_scalar(wpb[:, 0:G], psw[:, 0:G], float(b2),
                                        None, op0=OP.add)

                # msg for all chunks in one op: [P, G, 3] = rel * w
                relw_v = relw.rearrange("p (t f) -> p t f", f=4)
                msg = sm_pool.tile([P, 12], EDGE_DT, tag="msg")
                nc.vector.tensor_tensor(
                    msg[:, 0:3 * G].rearrange("p (c f) -> p c f", f=3),
                    relw_v[:, t0:t0 + G, 0:3],
                    wpb[:, 0:G, None].broadcast_to([P, G, 3]),
                    op=OP.mult)
                # one-hot for all chunks in one op: [P, G*128]
                oh = oh_pool.tile([P, 4 * P], EDGE_DT, tag="oh")
                nc.vector.tensor_tensor(
                    oh[:, 0:G * P].rearrange("p (c f) -> p c f", f=P),
                    iota[:, 0:G * P].rearrange("p (c f) -> p c f", f=P),
                    relw_v[:, t0:t0 + G, 3:4].broadcast_to([P, G, P]),
                    op=OP.is_equal)

                for c in range(G):
                    t = t0 + c
                    nt, j = divmod(t, K)
                    if j == 0:
                        ps_sc = pssc_pool.tile([P, 3], F32, tag="ps_sc")
                    nc.tensor.matmul(ps_sc[:], oh[:, c * P:(c + 1) * P],
                                     msg[:, 3 * c:3 * c + 3],
                                     start=(j == 0), stop=(j == K - 1))
                    if j == K - 1:
                        node_tail(nt, ps_sc)

            nc.sync.dma_start(out_d[:, :], outbuf[:])

    _split_excess_waits(nc)
    return nc


def _preprocess(inputs: dict):
    """Shard + lay out all per-core device inputs. Returns (in_maps, K, b2)."""
    h = np.asarray(inputs["h"], np.float32)
    m_ij = np.asarray(inputs["m_ij"], np.float32)
    x = np.asarray(inputs["x"], np.float32)
    vel_all = np.asarray(inputs["vel_all"], np.float32)
    ei = np.asarray(inputs["edge_index"])
    src = ei[0].astype(np.int64)
    dst = ei[1].astype(np.int64)

    counts = np.bincount(dst, minlength=N_NODES).astype(np.float32)
    invc = (1.0 / np.maximum(counts, 1.0)).astype(np.float32)

    order = np.argsort(dst, kind="stable")
    dst_s = dst[order]
    src_s = src[order]
    g = dst_s // P                       # global 128-node group, 0..391
    n_groups = N_PAD // P                # 392
    cg = np.bincount(g, minlength=n_groups)
    K = max(1, int(-(-cg.max() // P)))   # ceil(max group)/128
    ET = NT * K
    n_mac = (ET + 3) // 4
    slots_core = ET * P

    gstart = np.zeros(n_groups, np.int64)
    gstart[1:] = np.cumsum(cg)[:-1]
    within = np.arange(N_EDGES, dtype=np.int64) - gstart[g]
    slot = g * (K * P) + within          # slot in global [392, K*128] layout

    Sg = n_groups * K * P
    colidx = np.full(Sg, -1.0, np.float32)
    colidx[slot] = (dst_s % P).astype(np.float32)
    relp = np.zeros((Sg, 3), np.float32)
    relp[slot] = x[src_s] - x[dst_s]
    mijp = np.zeros((Sg, H), EDGE_NP)
    mijp[slot] = m_ij[order].astype(EDGE_NP)

    # padded node tensors
    hp = np.zeros((N_PAD, H), np.float32)
    hp[:N_NODES] = h
    velp = np.zeros((N_PAD, 5, 3), np.float32)
    velp[:N_NODES] = vel_all
    invp = np.ones(N_PAD, np.float32)
    invp[:N_NODES] = invc

    # weights (shared by all cores)
    w1 = np.asarray(inputs["ew_W1"], np.float32)
    b1 = np.asarray(inputs["ew_b1"], np.float32)
    w2 = np.asarray(inputs["ew_W2"], np.float32)
    b2 = float(np.asarray(inputs["ew_b2"], np.float32)[0])
    vgw1 = np.asarray(inputs["vg_W1"], np.float32)
    vgb1 = np.asarray(inputs["vg_b1"], np.float32)
    vgw2 = np.asarray(inputs["vg_W2"], np.float32)
    vgb2 = np.asarray(inputs["vg_b2"], np.float32)

    w1b = w1.reshape(2, P, 2, P).transpose(0, 2, 1, 3).astype(EDGE_NP).copy()
    w2t = w2.reshape(2, P, 1).astype(EDGE_NP).copy()
    b1t = b1.reshape(2, P, 1).copy()
    vgw1b = vgw1.reshape(2, P, 2, P).transpose(0, 2, 1, 3).astype(EDGE_NP).copy()
    vgw2t = vgw2.reshape(2, P, 5).astype(EDGE_NP).copy()
    vgb1t = vgb1.reshape(2, P, 1).copy()
    onesb2 = np.zeros((1, 133), EDGE_NP)
    onesb2[0, :P] = 1.0
    onesb2[0, P:P + 5] = vgb2.astype(EDGE_NP)

    mijp = mijp.reshape(N_CORES, ET, P, H)
    relp = relp.reshape(N_CORES, ET, P, 3)
    colidx = colidx.reshape(N_CORES, ET, P)

    n_mac = (ET + 3) // 4
    n_sup = (n_mac + 3) // 4
    in_maps = []
    for k in range(N_CORES):
        # mijT supertiles: [n_sup, 128, 4096]; col = mac*1024 + kk*512
        # + chunk*128 + e, partition = h within kk half
        b = mijp[k].transpose(0, 2, 1).reshape(ET, 2, P, P)
        full = np.zeros((n_sup * 16, 2, P, P), EDGE_NP)
        full[:ET] = b
        mijT = np.ascontiguousarray(
            full.reshape(n_sup, 4, 4, 2, P, P).transpose(0, 4, 1, 3, 2, 5)
        ).reshape(n_sup, P, 4096)

        rw = np.empty((P, ET, 4), np.float32)
        rw[:, :, 0:3] = relp[k].transpose(1, 0, 2)
        rw[:, :, 3] = colidx[k].T
        relw = np.ascontiguousarray(rw).reshape(P, ET * 4)

        hk = hp[k * NPC:(k + 1) * NPC].reshape(NT, P, H)
        hTk = np.ascontiguousarray(
            hk.transpose(0, 2, 1).reshape(NT, 2, P, P).transpose(0, 2, 1, 3)
        ).reshape(NT, P, 2 * P).astype(EDGE_NP)

        vg = np.empty((P, NT, 16), np.float32)
        vg[:, :, 0:15] = (velp[k * NPC:(k + 1) * NPC]
                          .reshape(NT, P, 15).transpose(1, 0, 2))
        vg[:, :, 15] = invp[k * NPC:(k + 1) * NPC].reshape(NT, P).T
        velg = np.ascontiguousarray(vg).reshape(P, NT * 16)

        in_maps.append({
            "mijT": mijT,
            "relw": relw,
            "hT": hTk,
            "velg": velg,
            "w1b": w1b,
            "w2t": w2t,
            "b1t": b1t,
            "vgw1b": vgw1b,
            "vgw2t": vgw2t,
            "vgb1t": vgb1t,
            "onesb2": onesb2,
        })
    return in_maps, K, b2


def unpack_out(arr: np.ndarray) -> np.ndarray:
    """[128, NT*3] packed per-core output -> [NPC, 3]."""
    return arr.reshape(P, NT, 3).transpose(1, 0, 2).reshape(NPC, 3)


def kernel(**inputs) -> np.ndarray:
    in_maps, K, b2 = _preprocess(inputs)
    nc = _build_program(K, b2)
    res = run_bass_kernel_spmd(nc, in_maps, list(range(N_CORES)))
    parts = [unpack_out(res.results[k]["out"]) for k in range(N_CORES)]
    return np.concatenate(parts, axis=0)[:N_NODES].astype(np.float32)


# revision 22
# speedup vs baseline: 1.8279x; 1.0719x over previous
"""Trainium2 Bass kernel for nn_EquivariantDecoder (GNN message passing).

Sharding: nodes are split into 8 contiguous ranges of 6272 (= 49 tiles of
128); each core owns the edges whose dst lands in its range, so per-node
segment sums are core-local (no collectives). Edges are sorted by dst on
the host and padded so every (core, node-tile) group holds exactly K
tiles of 128 edge slots; the K is baked into the traced program.

Device work per core:
  edge path:  w = silu(m_ij @ W1 + b1) @ W2 + b2   (tile-transposed m_ij)
              scatter-sum of rel*w into the 128-node tile via a one-hot
              matmul (one-hot built on device from dst%128 with iota +
              is_equal; padding slots use col=-1 so they vanish)
  node path:  alpha = silu(h @ vgW1 + vgb1) @ vgW2 + vgb2
              out = sum_k alpha_k * vel_k + scatter_sum * (1/max(cnt,1))
"""

import sys

import numpy as np

try:
    import concourse.bass as bass  # noqa: F401
except Exception:  # pragma: no cover
    sys.path.insert(0, "/opt/trn_rl_repo")

import concourse.bass as bass
import concourse.mybir as mybir
from concourse.bass_utils import run_bass_kernel_spmd
from concourse.tile import TileContext
from concourse.vector_clock import ScopedClock

N_NODES = 50000
N_EDGES = 800000
H = 256
N_CORES = 8
NT = 49                 # node tiles per core
NPC = NT * 128          # 6272 nodes per core
N_PAD = N_CORES * NPC   # 50176
P = 128

# edge-MLP matmul dtype: bfloat16 | float32r | float32
EDGE_DT = mybir.dt.bfloat16
EDGE_NP = mybir.dt.np(EDGE_DT)
F32 = mybir.dt.float32
AF = mybir.ActivationFunctionType
OP = mybir.AluOpType


# ---------------------------------------------------------------------------
# Walrus on this toolchain rejects >2 sync waits on the TileContext tail
# drain ("Too many sync wait commands"); split them across SP NOPs.
def _patched_drain_and_barrier(self, tick_clock, wait_clock):
    drain_inst = self.nc.sync.drain()
    wait_clock.add_sem_waits(
        drain_inst.ins, ScopedClock({None: tick_clock.global_clock})
    )
    si = drain_inst.ins.sync_info
    if si is not None and si.on_wait and len(si.on_wait) > 1:
        extra = list(si.on_wait[1:])
        del si.on_wait[1:]
        for w in extra:
            nop = self.nc.sync.nop(nofuse=True, hint="drain_wait_split")
            nsi = nop.ins.sync_info
            if nsi is None:
                nop.ins.sync_info = mybir.SyncInfo(on_wait=[w], on_update=[])
            else:
                nsi.on_wait.append(w)

    self.nc.all_engine_barrier()
    assert self.sems is not None
    popped = self.nc._tile_sem_poison_stack.pop()
    assert popped is self._sem_poison
    self.nc.clear_and_free_semaphores(list(self.sems.allocated().values()))
    self.nc.all_engine_barrier()


TileContext._drain_and_barrier = _patched_drain_and_barrier


def _split_excess_waits(nc, maxw: int = 1):
    """Walrus rejects >maxw sync waits on one instruction; move the excess
    onto NOPs inserted just before, on the same engine (same-queue program
    order makes this equivalent)."""
    n_split = 0
    for f in nc.m.functions:
        for b in f.blocks:
            out = []
            for inst in b.instructions:
                si = inst.sync_info
                if si is not None and si.on_wait and len(si.on_wait) > maxw:
                    extra = list(si.on_wait[: -maxw])
                    del si.on_wait[: -maxw]
                    for i in range(0, len(extra), maxw):
                        nop = mybir.InstNoOp(
                            name=f"{inst.name}-wsplit{i}",
                            engine=inst.engine,
                            sync_info=mybir.SyncInfo(
                                on_wait=extra[i:i + maxw], on_update=[]),
                            bass_nofuse=True,
                        )
                        out.append(nop)
                    n_split += 1
                out.append(inst)
            b.instructions[:] = out
    return n_split
# ---------------------------------------------------------------------------


def _build_program(K: int, b2: float):
    """Trace the single-core SPMD program for a fixed K (edge tiles per
    node-tile group)."""
    ET = NT * K                      # edge tiles per core
    n_mac = (ET + 3) // 4            # macros of up to 4 edge tiles
    n_sup = (n_mac + 3) // 4         # supertiles of 4 macros (1 DMA each)

    nc = bass.Bass()

    mijT = nc.dram_tensor("mijT", [n_sup, P, 4096], EDGE_DT, kind="ExternalInput")
    relw_d = nc.dram_tensor("relw", [P, ET * 4], F32, kind="ExternalInput")
    hT = nc.dram_tensor("hT", [NT, P, 2 * P], EDGE_DT, kind="ExternalInput")
    velg_d = nc.dram_tensor("velg", [P, NT * 16], F32, kind="ExternalInput")
    w1b_d = nc.dram_tensor("w1b", [2, 2, P, P], EDGE_DT, kind="ExternalInput")
    w2t_d = nc.dram_tensor("w2t", [2, P, 1], EDGE_DT, kind="ExternalInput")
    b1t_d = nc.dram_tensor("b1t", [2, P, 1], F32, kind="ExternalInput")
    vgw1b_d = nc.dram_tensor("vgw1b", [2, 2, P, P], EDGE_DT, kind="ExternalInput")
    vgw2t_d = nc.dram_tensor("vgw2t", [2, P, 5], EDGE_DT, kind="ExternalInput")
    vgb1t_d = nc.dram_tensor("vgb1t", [2, P, 1], F32, kind="ExternalInput")
    onesb2_d = nc.dram_tensor("onesb2", [1, 133], EDGE_DT, kind="ExternalInput")
    out_d = nc.dram_tensor("out", [P, NT * 3], F32, kind="ExternalOutput")

    with TileContext(nc) as tc:
        with (
            tc.tile_pool(name="const", bufs=1) as cpool,
            tc.tile_pool(name="rhs", bufs=3) as rhs_pool,
            tc.tile_pool(name="s1", bufs=3) as s1_pool,
            tc.tile_pool(name="small", bufs=6) as sm_pool,
            tc.tile_pool(name="oh", bufs=4) as oh_pool,
            tc.tile_pool(name="nodes", bufs=3) as nd_pool,
            tc.tile_pool(name="ps_mm1", bufs=3, space="PSUM") as ps1_pool,
            tc.tile_pool(name="ps_w", bufs=2, space="PSUM") as psw_pool,
            tc.tile_pool(name="ps_sc", bufs=2, space="PSUM") as pssc_pool,
        ):
            # ---- constants ----
            w1 = [[cpool.tile([P, P], EDGE_DT, tag=f"w1_{kk}{hh}", name=f"w1_{kk}{hh}")
                   for hh in range(2)] for kk in range(2)]
            vgw1 = [[cpool.tile([P, P], EDGE_DT, tag=f"vgw1_{kk}{hh}", name=f"vgw1_{kk}{hh}")
                     for hh in range(2)] for kk in range(2)]
            for kk in range(2):
                for hh in range(2):
                    nc.sync.dma_start(w1[kk][hh][:], w1b_d[kk, hh, :, :])
                    nc.sync.dma_start(vgw1[kk][hh][:], vgw1b_d[kk, hh, :, :])
            w2 = [cpool.tile([P, 1], EDGE_DT, tag=f"w2_{hh}", name=f"w2_{hh}") for hh in range(2)]
            b1 = [cpool.tile([P, 1], F32, tag=f"b1_{hh}", name=f"b1_{hh}") for hh in range(2)]
            vgw2 = [cpool.tile([P, 5], EDGE_DT, tag=f"vgw2_{hh}", name=f"vgw2_{hh}") for hh in range(2)]
            vgb1 = [cpool.tile([P, 1], F32, tag=f"vgb1_{hh}", name=f"vgb1_{hh}") for hh in range(2)]
            for hh in range(2):
                nc.sync.dma_start(w2[hh][:], w2t_d[hh, :, :])
                nc.sync.dma_start(b1[hh][:], b1t_d[hh, :, :])
                nc.sync.dma_start(vgw2[hh][:], vgw2t_d[hh, :, :])
                nc.sync.dma_start(vgb1[hh][:], vgb1t_d[hh, :, :])
            onesb2 = cpool.tile([1, 133], EDGE_DT, tag="onesb2")
            nc.sync.dma_start(onesb2[:], onesb2_d[0, :][None, :])

            # iota values 0..127 repeated 4x: one-hot build for a whole macro
            iota = cpool.tile([P, 4 * P], EDGE_DT, tag="iota")
            nc.gpsimd.iota(iota[:], pattern=[[0, 4], [1, P]], base=0,
                           channel_multiplier=0,
                           allow_small_or_imprecise_dtypes=True)

            relw = cpool.tile([P, ET * 4], F32, tag="relw")
            nc.scalar.dma_start(relw[:], relw_d[:, :])
            velg = cpool.tile([P, NT * 16], F32, tag="velg")
            nc.scalar.dma_start(velg[:], velg_d[:, :])

            # packed output, one column triple per node tile; single DMA at end
            outbuf = cpool.tile([P, NT * 3], F32, tag="outbuf")

            # node-tail emission, called when node tile nt's scatter psum is
            # fully accumulated
            def node_tail(nt: int, ps_sc):
                geom = sm_pool.tile([P, 3], F32, tag="geom")
                nc.vector.tensor_scalar(
                    geom[:], ps_sc[:, 0:3], velg[:, nt * 16 + 15:nt * 16 + 16],
                    None, op0=OP.mult)

                rhn = nd_pool.tile([P, 2 * P], EDGE_DT, tag="rhn")
                eng = nc.sync if nt % 2 == 0 else nc.scalar
                eng.dma_start(rhn[:], hT[nt, :, :])
                psn = [ps1_pool.tile([P, P], F32, tag="ps_mm1", name="psn")
                       for _ in range(2)]
                for hh in range(2):
                    for kk in range(2):
                        nc.tensor.matmul(psn[hh][:], vgw1[kk][hh][:],
                                         rhn[:, kk * P:(kk + 1) * P],
                                         start=(kk == 0), stop=(kk == 1))
                s1n = [nd_pool.tile([P, P], EDGE_DT, tag=f"s1n_{hh}", name=f"s1n_{hh}")
                       for hh in range(2)]
                for hh in range(2):
                    nc.scalar.activation(s1n[hh][:], psn[hh][:], AF.Silu,
                                         bias=vgb1[hh][:, 0:1], scale=1.0)
                psa = psw_pool.tile([P, 8], F32, tag="ps_w")
                for hh in range(2):
                    nc.tensor.matmul(psa[:, 0:5], s1n[hh][:], vgw2[hh][:],
                                     start=(hh == 0), stop=False)
                nc.tensor.matmul(psa[:, 0:5], onesb2[:, 0:128],
                                 onesb2[:, 128:133], start=False, stop=True)

                # out[:, j] = geom[:, j] + sum_k alpha[k] * vel[j, k]
                scratch = sm_pool.tile([P, 15], F32, tag="scratch")
                vbase = nt * 16
                velg_v = velg[:, vbase:vbase + 15].rearrange(
                    "p (j k) -> p j k", k=5)
                nc.vector.tensor_tensor(
                    scratch[:].rearrange("p (j k) -> p j k", k=5),
                    velg_v,
                    psa[:, None, 0:5].broadcast_to([P, 3, 5]),
                    op=OP.mult)
                acc = sm_pool.tile([P, 3], F32, tag="acc")
                nc.vector.tensor_reduce(
                    acc[:, :, None],
                    scratch[:].rearrange("p (j k) -> p j k", k=5),
                    axis=mybir.AxisListType.X, op=OP.add)
                nc.vector.tensor_add(outbuf[:, nt * 3:nt * 3 + 3],
                                     acc[:], geom[:])

            # ---- edge-path macro loop ----
            ps_sc = None
            sup = None
            for m in range(n_mac):
                t0 = m * 4
                G = min(4, ET - t0)          # real edge tiles in this macro
                si, sm = divmod(m, 4)
                if sm == 0:
                    sup = rhs_pool.tile([P, 4096], EDGE_DT, tag="sup",
                                        name="sup")
                    eng = nc.sync if si % 2 == 0 else nc.scalar
                    eng.dma_start(sup[:], mijT[si, :, :])
                rhs = sup[:, sm * 1024:(sm + 1) * 1024]
                ps1 = [ps1_pool.tile([P, 512], F32, tag="ps_mm1", name="ps1")
                       for _ in range(2)]
                for hh in range(2):
                    for kk in range(2):
                        nc.tensor.matmul(ps1[hh][:], w1[kk][hh][:],
                                         rhs[:, kk * 512:(kk + 1) * 512],
                                         start=(kk == 0), stop=(kk == 1))
                s1 = [s1_pool.tile([P, 512], EDGE_DT, tag=f"s1_{hh}", name=f"s1_{hh}")
                      for hh in range(2)]
                for hh in range(2):
                    nc.scalar.activation(s1[hh][:], ps1[hh][:], AF.Silu,
                                         bias=b1[hh][:, 0:1], scale=1.0)
                psw = psw_pool.tile([P, 8], F32, tag="ps_w")
                for c in range(G):
                    for hh in range(2):
                        nc.tensor.matmul(psw[:, c:c + 1],
                                         s1[hh][:, c * P:(c + 1) * P],
                                         w2[hh][:],
                                         start=(hh == 0), stop=(hh == 1))

                # msg for all chunks in one op: [P, G, 3] = (w + b2) * rel
                relw_v = relw.rearrange("p (t f) -> p t f", f=4)
                msg = sm_pool.tile([P, 12], EDGE_DT, tag="msg")
                nc.vector.scalar_tensor_tensor(
                    msg[:, 0:3 * G].rearrange("p (c f) -> p c f", f=3),
                    psw[:, 0:G, None].broadcast_to([P, G, 3]),
                    float(b2),
                    relw_v[:, t0:t0 + G, 0:3],
                    op0=OP.add, op1=OP.mult)
                # one-hot for all chunks in one op: [P, G*128]
                oh = oh_pool.tile([P, 4 * P], EDGE_DT, tag="oh")
                nc.vector.tensor_tensor(
                    oh[:, 0:G * P].rearrange("p (c f) -> p c f", f=P),
                    iota[:, 0:G * P].rearrange("p (c f) -> p c f", f=P),
                    relw_v[:, t0:t0 + G, 3:4].broadcast_to([P, G, P]),
                    op=OP.is_equal)

                for c in range(G):
                    t = t0 + c
                    nt, j = divmod(t, K)
                    if j == 0:
                        ps_sc = pssc_pool.tile([P, 3], F32, tag="ps_sc")
                    nc.tensor.matmul(ps_sc[:], oh[:, c * P:(c + 1) * P],
                                     msg[:, 3 * c:3 * c + 3],
                                     start=(j == 0), stop=(j == K - 1))
                    if j == K - 1:
                        node_tail(nt, ps_sc)

            nc.sync.dma_start(out_d[:, :], outbuf[:])

    _split_excess_waits(nc)
    return nc


def _preprocess(inputs: dict):
    """Shard + lay out all per-core device inputs. Returns (in_maps, K, b2)."""
    h = np.asarray(inputs["h"], np.float32)
    m_ij = np.asarray(inputs["m_ij"], np.float32)
    x = np.asarray(inputs["x"], np.float32)
    vel_all = np.asarray(inputs["vel_all"], np.float32)
    ei = np.asarray(inputs["edge_index"])
    src = ei[0].astype(np.int64)
    dst = ei[1].astype(np.int64)

    counts = np.bincount(dst, minlength=N_NODES).astype(np.float32)
    invc = (1.0 / np.maximum(counts, 1.0)).astype(np.float32)

    order = np.argsort(dst, kind="stable")
    dst_s = dst[order]
    src_s = src[order]
    g = dst_s // P                       # global 128-node group, 0..391
    n_groups = N_PAD // P                # 392
    cg = np.bincount(g, minlength=n_groups)
    K = max(1, int(-(-cg.max() // P)))   # ceil(max group)/128
    ET = NT * K
    n_mac = (ET + 3) // 4
    slots_core = ET * P

    gstart = np.zeros(n_groups, np.int64)
    gstart[1:] = np.cumsum(cg)[:-1]
    within = np.arange(N_EDGES, dtype=np.int64) - gstart[g]
    slot = g * (K * P) + within          # slot in global [392, K*128] layout

    Sg = n_groups * K * P
    colidx = np.full(Sg, -1.0, np.float32)
    colidx[slot] = (dst_s % P).astype(np.float32)
    relp = np.zeros((Sg, 3), np.float32)
    relp[slot] = x[src_s] - x[dst_s]
    mijp = np.zeros((Sg, H), EDGE_NP)
    mijp[slot] = m_ij[order].astype(EDGE_NP)

    # padded node tensors
    hp = np.zeros((N_PAD, H), np.float32)
    hp[:N_NODES] = h
    velp = np.zeros((N_PAD, 5, 3), np.float32)
    velp[:N_NODES] = vel_all
    invp = np.ones(N_PAD, np.float32)
    invp[:N_NODES] = invc

    # weights (shared by all cores)
    w1 = np.asarray(inputs["ew_W1"], np.float32)
    b1 = np.asarray(inputs["ew_b1"], np.float32)
    w2 = np.asarray(inputs["ew_W2"], np.float32)
    b2 = float(np.asarray(inputs["ew_b2"], np.float32)[0])
    vgw1 = np.asarray(inputs["vg_W1"], np.float32)
    vgb1 = np.asarray(inputs["vg_b1"], np.float32)
    vgw2 = np.asarray(inputs["vg_W2"], np.float32)
    vgb2 = np.asarray(inputs["vg_b2"], np.float32)

    w1b = w1.reshape(2, P, 2, P).transpose(0, 2, 1, 3).astype(EDGE_NP).copy()
    w2t = w2.reshape(2, P, 1).astype(EDGE_NP).copy()
    b1t = b1.reshape(2, P, 1).copy()
    vgw1b = vgw1.reshape(2, P, 2, P).transpose(0, 2, 1, 3).astype(EDGE_NP).copy()
    vgw2t = vgw2.reshape(2, P, 5).astype(EDGE_NP).copy()
    vgb1t = vgb1.reshape(2, P, 1).copy()
    onesb2 = np.zeros((1, 133), EDGE_NP)
    onesb2[0, :P] = 1.0
    onesb2[0, P:P + 5] = vgb2.astype(EDGE_NP)

    mijp = mijp.reshape(N_CORES, ET, P, H)
    relp = relp.reshape(N_CORES, ET, P, 3)
    colidx = colidx.reshape(N_CORES, ET, P)

    n_mac = (ET + 3) // 4
    n_sup = (n_mac + 3) // 4
    in_maps = []
    for k in range(N_CORES):
        # mijT supertiles: [n_sup, 128, 4096]; col = mac*1024 + kk*512
        # + chunk*128 + e, partition = h within kk half
        b = mijp[k].transpose(0, 2, 1).reshape(ET, 2, P, P)
        full = np.zeros((n_sup * 16, 2, P, P), EDGE_NP)
        full[:ET] = b
        mijT = np.ascontiguousarray(
            full.reshape(n_sup, 4, 4, 2, P, P).transpose(0, 4, 1, 3, 2, 5)
        ).reshape(n_sup, P, 4096)

        rw = np.empty((P, ET, 4), np.float32)
        rw[:, :, 0:3] = relp[k].transpose(1, 0, 2)
        rw[:, :, 3] = colidx[k].T
        relw = np.ascontiguousarray(rw).reshape(P, ET * 4)

        hk = hp[k * NPC:(k + 1) * NPC].reshape(NT, P, H)
        hTk = np.ascontiguousarray(
            hk.transpose(0, 2, 1).reshape(NT, 2, P, P).transpose(0, 2, 1, 3)
        ).reshape(NT, P, 2 * P).astype(EDGE_NP)

        # velg cols per node tile: [comp j, gate k] at 5*j+k, inv_count at 15
        vg = np.empty((P, NT, 16), np.float32)
        vg[:, :, 0:15] = (velp[k * NPC:(k + 1) * NPC]
                          .reshape(NT, P, 5, 3).transpose(1, 0, 3, 2)
                          .reshape(P, NT, 15))
        vg[:, :, 15] = invp[k * NPC:(k + 1) * NPC].reshape(NT, P).T
        velg = np.ascontiguousarray(vg).reshape(P, NT * 16)

        in_maps.append({
            "mijT": mijT,
            "relw": relw,
            "hT": hTk,
            "velg": velg,
            "w1b": w1b,
            "w2t": w2t,
            "b1t": b1t,
            "vgw1b": vgw1b,
            "vgw2t": vgw2t,
            "vgb1t": vgb1t,
            "onesb2": onesb2,
        })
    return in_maps, K, b2


def unpack_out(arr: np.ndarray) -> np.ndarray:
    """[128, NT*3] packed per-core output -> [NPC, 3]."""
    return arr.reshape(P, NT, 3).transpose(1, 0, 2).reshape(NPC, 3)


def kernel(**inputs) -> np.ndarray:
    in_maps, K, b2 = _preprocess(inputs)
    nc = _build_program(K, b2)
    res = run_bass_kernel_spmd(nc, in_maps, list(range(N_CORES)))
    parts = [unpack_out(res.results[k]["out"]) for k in range(N_CORES)]
    return np.concatenate(parts, axis=0)[:N_NODES].astype(np.float32)
